# revision 20
# baseline (speedup 1.0000x reference)
"""MHSA (dense transformer, ALiBi + causal) TRN2 Bass kernel, 8-core SPMD.

Sharding: batch (2) x head-group (4 groups of 4 heads) -> 8 cores.
Head groups are chosen so every core gets one head from each ALiBi band
class; per-head causal attention is BANDED: with slope s_h, weights decay
like e^{s_h (j-i)}, so blocks further than nb_h*128 below the diagonal
contribute negligible mass and are skipped. Band slots (in local head
order) are [16, 6, 3, 2] diagonal 128-blocks, identical on every core.

Per core, for its batch b and 4 heads:
  Phase 1 (t-major, K before Q):
    K^T = Wk_g @ X_k^T then Q^T = (Wq_g/8) @ X_q^T  (bf16 matmuls, f32
    psum). Per 256-col t-block, 8 contraction rounds accumulate in psum;
    per t-half the psum is split to per-head f32r rows of q~/k~ with 2
    exact aug rows folding ALiBi into the S contraction:
       q~ = [q; slope_h; -slope_h*i],  k~ = [k; j; 1]
    Even head slots hold data rows 0:64 + aug 64:66 (contract 0:66);
    odd slots hold aug 62:64 + data 64:128 (contract 62:128) -- no
    zero fill needed.
  S^T[j, i] banded, streamed into psum windows; "early" windows (i and
    j in the first t-half) only need K fully + Q half 0, so their
    exp(S - 12) (ScalarE) starts while Q half 1 is still projecting.
    Diagonal 128-blocks masked with a lower-triangular multiply on DVE.
  V = X_v @ Wv_g^T -> v~[j, d] bf16 with a ones column (row-sum ->
    softmax denominator); computed per tt-pair from t-major xv blocks.
  O[i, d] = sum_J pt_J^T @ v~_J : psum [128 i, 6 I-blocks, 65]; col 64
    is the denominator. I-major inside a group (a matmul's start clears
    the whole psum bank's has_written bits). Per-group epilogue: DVE
    reciprocal + per-partition scale -> bf16 out_sb, then a chunked
    output DMA ([128, 384] to a p-major DRAM layout, >=512B runs).

Emission interleaves S windows with V-projection and O groups driven by
cost-model time estimates so PE / ACT / DMA all stay busy.
"""

import numpy as np
import ml_dtypes

import concourse.bass as bass
import concourse.mybir as mybir
import concourse.tile as tile
from concourse import bacc
from concourse.bass_utils import run_bass_kernel_spmd

P = 128
S = 2048
D = 1024
H = 16
HWID = 64
HPC = 4            # heads per core
CW = HPC * HWID    # 256 weight/output cols per core
NKC = D // P       # 8 contraction chunks
NJ = S // P        # 16 j/i blocks of 128
HS = S // 2        # t-half
NTB = 8            # 256-col t-blocks
TBW = S // NTB     # 256

F32 = mybir.dt.float32
F32R = mybir.dt.float32r
BF16 = mybir.dt.bfloat16

EXP_BIAS = -12.0

# ALiBi band widths (in 128-blocks, incl. diagonal) per local head slot.
NB_SLOT = [16, 6, 3, 2]
# global head ids per (group, slot): slot0 gets the widest-band heads.
GROUPS = [[13, 11, 7, 3], [14, 10, 6, 2], [15, 9, 5, 1], [12, 8, 4, 0]]

GSZ = 6            # I-blocks per O psum group
NGRP = (NJ + GSZ - 1) // GSZ   # 3 groups: I 0-5, 6-11, 12-15

# ---------------------------------------------------------------------------
# Segment / window planning.
#
# A "segment" is (J, i0, w): S^T rows j in block J, query cols i in
# [i0, i0+w).  Early segments have J < 8 and i < 1024 (computable after
# K full + Q half 0); late segments are the rest.  Segments are packed
# (in order) into psum windows; matmul chunks split at 512-col psum bank
# boundaries.  pt columns are assigned contiguously per head:
# all early segments first, then all late segments.
# ---------------------------------------------------------------------------


def _plan():
    plans = []
    for nb in NB_SLOT:
        segs_e, segs_l = [], []
        for J in range(NJ):
            ihi = min(J + nb, NJ) * P
            ilo = J * P
            if J < NJ // 2:
                w_e = min(ihi, HS) - ilo
                if w_e > 0:
                    segs_e.append((J, ilo, w_e))
                if ihi > HS:
                    segs_l.append((J, HS, ihi - HS))
            else:
                segs_l.append((J, ilo, ihi - ilo))

        # pack segments into windows; returns (wins, offs, total_width)
        def pack(segs, winw, pt0):
            wins = []
            offs = {}
            cur = 0
            ptbase = pt0
            chunks, diags = [], []
            ptw = pt0

            def close():
                nonlocal cur, ptbase, chunks, diags
                if cur > 0:
                    wins.append((cur, ptbase, chunks, diags))
                    ptbase += cur
                chunks, diags = [], []
                cur = 0

            for (J, i0, w) in segs:
                if cur == winw:
                    close()
                offs[(J, i0)] = ptw
                if J * P == i0:
                    diags.append(ptw)       # diagonal block starts segment
                done = 0
                while done < w:
                    if cur == winw:
                        close()
                    c = min(w - done, 512 - (cur % 512), winw - cur)
                    chunks.append((J, i0 + done, cur, c))
                    done += c
                    cur += c
                    ptw += c
            close()
            return wins, offs, ptw

        wins_e, offs_e, pt_mid = pack(segs_e, 1024, 0)
        wins_l, offs_l, pt_end = pack(segs_l, 1536, pt_mid)
        offs = dict(offs_e)
        offs.update(offs_l)
        plans.append({
            "nb": nb, "wins_e": wins_e, "wins_l": wins_l,
            "offs": offs, "ptw": pt_end,
        })
    return plans


PLANS = _plan()


def _o_col(h, J, I):
    """pt column of the (J, I) 128-block for head slot h."""
    offs = PLANS[h]["offs"]
    if I < NJ // 2:
        base_i = J * P
    else:
        base_i = HS if J < NJ // 2 else J * P
    return offs[(J, base_i)] + (I * P - base_i)


# ---------------------------------------------------------------------------
# Cost-model constants for the emission-time schedule estimates (ns).
# ---------------------------------------------------------------------------
PEC = 1.0 / 2.4            # ns per matmul output row at full clock
ACTC = 1.0 / 1.2
DVEC = 1.0 / 0.96
EXP_OVH = 190.0
COPY_OVH = 280.0
DVE_OVH = 160.0
HWDGE_NS = 625.0
DGE_DELAY = 650.0
SEM_DMA = 900.0
BUS = 360.0                # bytes per ns


def build_kernel():
    nc = bacc.Bacc("TRN2")

    xq = nc.dram_tensor("xq", [D, S], BF16, kind="ExternalInput")
    xk = nc.dram_tensor("xk", [D, S], BF16, kind="ExternalInput")
    xv = nc.dram_tensor("xv", [D, S], BF16, kind="ExternalInput")
    wq = nc.dram_tensor("wq", [D, CW], BF16, kind="ExternalInput")
    wk = nc.dram_tensor("wk", [D, CW], BF16, kind="ExternalInput")
    wv = nc.dram_tensor("wv", [D, CW], BF16, kind="ExternalInput")
    # aug[t][r][s][:]: t: 0=q,1=k; r: aug row; s: head slot
    aug = nc.dram_tensor("aug", [2, 2, HPC, S], F32R, kind="ExternalInput")
    tri = nc.dram_tensor("tri", [P, P], BF16, kind="ExternalInput")
    outs_d = [
        nc.dram_tensor(f"out{h}", [P, NJ * HWID], BF16, kind="ExternalOutput")
        for h in range(HPC)
    ]

    # ---- emission-time schedule estimates ----
    est = {"pe": 0.0, "act": 0.0, "dve": 0.0}
    dma = {"hwdge": 0.0, "bus": 0.0}
    arr = {}   # name -> ready estimate

    def dma_est(name, nbytes, small=False):
        dma["hwdge"] += HWDGE_NS
        start = max(dma["bus"], dma["hwdge"] + DGE_DELAY)
        dma["bus"] = start + nbytes * (2.0 if small else 1.0) / BUS
        arr[name] = dma["bus"] + SEM_DMA

    with tile.TileContext(nc) as tc:
        with (
            tc.tile_pool(name="qk", bufs=1) as qkp,
            tc.tile_pool(name="vv", bufs=1) as vvp,
            tc.tile_pool(name="ob", bufs=1) as obp,
            tc.tile_pool(name="rc", bufs=4) as rcp,
        ):
            # ---- long-lived SBUF ----
            # rows 0:64 head data, 64:66 aug; every head contracts 0:66
            q_t = qkp.tile([HWID + 2, HPC, S], F32R, tag="qt", name="q_t")
            k_t = qkp.tile([HWID + 2, HPC, S], F32R, tag="kt", name="k_t")
            v_sb = vvp.tile([P, NJ, HPC, HWID + 1], BF16, tag="v", name="v_sb")
            out_sb = [
                obp.tile([P, NJ, HWID], BF16, tag=f"ob{h}", name=f"ob{h}")
                for h in range(HPC)
            ]
            ebias = vvp.tile([P, 1], F32, tag="ebias", name="ebias")
            tri_t = vvp.tile([P, P], BF16, tag="tri", name="tri_t")
            wrm = vvp.tile([P, 512], BF16, tag="wrm", name="wrm")

            # ---- scoped x/w pools ----
            x2p = tc.alloc_tile_pool(name="x2", bufs=1, space="SBUF")   # wv, xv
            x1p = tc.alloc_tile_pool(name="x1", bufs=1, space="SBUF")   # wq/wk/xq/xk

            wvt = x2p.tile([P, NKC, CW], BF16, tag="wv", name="wv_t")
            xv_b = [x2p.tile([P, NKC, TBW], BF16, tag=f"xv{t}", name=f"xv{t}")
                    for t in range(NTB)]
            wqt = x1p.tile([P, NKC, CW], BF16, tag="wq", name="wq_t")
            wkt = x1p.tile([P, NKC, CW], BF16, tag="wk", name="wk_t")
            xq_b = [x1p.tile([P, NKC, TBW], BF16, tag=f"xq{t}", name=f"xq{t}")
                    for t in range(NTB)]
            xk_b = [x1p.tile([P, NKC, TBW], BF16, tag=f"xk{t}", name=f"xk{t}")
                    for t in range(NTB)]

            # ---- the full input DMA stream, in arrival-priority order ----
            def xdma(name, dst, src_dram, tb, ko_lo, ko_hi):
                nc.sync.dma_start(
                    dst[:, ko_lo:ko_hi, :],
                    src_dram.rearrange("(ko p) (tb c) -> tb p ko c",
                                       p=P, c=TBW)[tb, :, ko_lo:ko_hi, :])
                dma_est(name, (ko_hi - ko_lo) * P * TBW * 2)

            nc.sync.dma_start(wkt[:, 0:1, :], wk.rearrange(
                "(ko p) c -> p ko c", p=P)[:, 0:1, :])
            dma_est("wk0", P * CW * 2)
            xdma("xk0a", xk_b[0], xk, 0, 0, 1)
            xdma("xk0b", xk_b[0], xk, 0, 1, NKC)
            nc.sync.dma_start(wkt[:, 1:NKC, :], wk.rearrange(
                "(ko p) c -> p ko c", p=P)[:, 1:NKC, :])
            dma_est("wkR", (NKC - 1) * P * CW * 2)
            # aug rows + tri early (tiny), before the big stream
            for t, dst in enumerate((q_t, k_t)):
                nc.sync.dma_start(dst[HWID:HWID + 2, :, :], aug[t])
                dma_est(f"aug{t}", 2 * HPC * S * 4)
            nc.sync.dma_start(tri_t[:], tri[:])
            dma_est("tri", P * P * 2, small=True)
            for tb in range(1, NTB):
                xdma(f"xk{tb}", xk_b[tb], xk, tb, 0, NKC)
            nc.sync.dma_start(wqt[:], wq.rearrange("(ko p) c -> p ko c", p=P))
            dma_est("wq", D * CW * 2)
            for tb in range(NTB):
                xdma(f"xq{tb}", xq_b[tb], xq, tb, 0, NKC)
            nc.sync.dma_start(wvt[:], wv.rearrange("(ko p) c -> p ko c", p=P))
            dma_est("wv", D * CW * 2)
            for tb in range(NTB):
                xdma(f"xv{tb}", xv_b[tb], xv, tb, 0, NKC)

            # ---- on-chip constants (Pool) + PE warmup ----
            nc.gpsimd.memset(ebias[:], EXP_BIAS)
            nc.gpsimd.memset(wrm[:], 0.25)
            nc.gpsimd.memset(v_sb[:, :, :, HWID], 1.0)

            wrmp = tc.alloc_tile_pool(name="wp", bufs=1, space="PSUM")
            wps = wrmp.tile([P, 512], F32, tag="wp", name="wps")
            for i in range(4):
                nc.tensor.matmul(wps[:], lhsT=wrm[:, 0:P], rhs=wrm[:],
                                 start=True, stop=True)
            wrmp.release()
            est["pe"] = 2000.0

            # ---------- phase 1: K then Q, t-major ----------
            # psum: 4 tiles [64, 1024] (one per head slot), reused across
            # t-halves; every head's data lands on psum rows 0:64.
            def mm_slice(h):
                return slice(0, HWID + 2)

            def proj_phase(wt, xs, dst, xname):
                pp = tc.alloc_tile_pool(name="pp", bufs=1, space="PSUM")
                pst = [pp.tile([HWID, 1024], F32, tag=f"p{sl}",
                               name=f"p_{xname}{sl}")
                       for sl in range(HPC)]
                for hf in range(2):
                    for tbl in range(4):
                        tb = hf * 4 + tbl
                        # wait for this x block
                        est["pe"] = max(est["pe"], arr.get(f"{xname}{tb}",
                                                           arr.get(f"{xname}{tb}a", 0.0)))
                        for kk in range(NKC):
                            if tb == 0 and xname == "xk" and kk == 1:
                                est["pe"] = max(est["pe"], arr["xk0b"])
                            for sl in range(HPC):
                                nc.tensor.matmul(
                                    pst[sl][:, tbl * TBW:(tbl + 1) * TBW],
                                    lhsT=wt[:, kk, sl * HWID:(sl + 1) * HWID],
                                    rhs=xs[tb][:, kk, :],
                                    start=(kk == 0), stop=(kk == NKC - 1))
                            est["pe"] += HPC * TBW * PEC
                    # copies for this half: slots 0,1 -> ACT, 2,3 -> DVE
                    tsl = slice(hf * HS, (hf + 1) * HS)
                    for sl in range(HPC):
                        if sl < 2:
                            nc.scalar.copy(dst[0:HWID, sl, tsl], pst[sl][:])
                            est["act"] = (max(est["act"], est["pe"])
                                          + 1024 * ACTC + COPY_OVH)
                        else:
                            nc.vector.tensor_copy(dst[0:HWID, sl, tsl],
                                                  pst[sl][:])
                            est["dve"] = (max(est["dve"], est["pe"])
                                          + 1024 * DVEC + COPY_OVH)
                pp.release()

            proj_phase(wkt, xk_b, k_t, "xk")
            proj_phase(wqt, xq_b, q_t, "xq")
            x1p.release()

            # ---------- attention ----------
            ptp = tc.alloc_tile_pool(name="pt", bufs=1, space="SBUF")
            pt_t = [ptp.tile([P, PLANS[h]["ptw"]], BF16, tag=f"pt{h}",
                             name=f"pt{h}")
                    for h in range(HPC)]

            scp = tc.alloc_tile_pool(name="sc", bufs=1, space="PSUM")
            sc_t = [scp.tile([P, 1536], F32, tag=f"sc{i}", name=f"sc{i}")
                    for i in range(2)]
            ppV = tc.alloc_tile_pool(name="pv", bufs=1, space="PSUM")
            vps = ppV.tile([P, 512], F32, tag="pv", name="vps")
            ppO = tc.alloc_tile_pool(name="po", bufs=1, space="PSUM")
            ops = ppO.tile([P, GSZ, HWID + 1], F32, tag="po", name="ops")

            # window queue: (h, win, is_early) in emission order
            win_q = []
            for h in range(HPC):
                for w in PLANS[h]["wins_e"]:
                    win_q.append((h, w))
            for h in range(HPC):
                for w in PLANS[h]["wins_l"]:
                    win_q.append((h, w))

            # est completion of exp per (h, pt_col_window) for O gating
            exp_done = {}   # (h, window ptbase) -> act est
            win_ranges = {h: [(ptb, ptb + wd)
                              for (wd, ptb, _c, _d) in
                              PLANS[h]["wins_e"] + PLANS[h]["wins_l"]]
                          for h in range(HPC)}

            def win_base_of(h, col):
                for (lo, hi) in win_ranges[h]:
                    if lo <= col < hi:
                        return lo
                raise AssertionError((h, col))

            sidx = [0]
            sc_free = [0.0, 0.0]   # act-done est per sc tile (psum reuse)
            vps_free = [0.0]
            ops_free = [0.0]

            def emit_win(hw_):
                h, (width, ptbase, chunks, diags) = hw_
                sl = mm_slice(h)
                w_t = sc_t[sidx[0] % 2]
                est["pe"] = max(est["pe"], sc_free[sidx[0] % 2])
                pe = 0.0
                for (J, qc, wo, w) in chunks:
                    nc.tensor.matmul(
                        w_t[:, wo:wo + w],
                        lhsT=k_t[sl, h, J * P:(J + 1) * P],
                        rhs=q_t[sl, h, qc:qc + w],
                        start=True, stop=True)
                    pe += w * PEC * (4.0 if w < 256 else 1.0)
                est["pe"] += pe
                nc.scalar.activation(
                    pt_t[h][:, ptbase:ptbase + width], w_t[:, 0:width],
                    mybir.ActivationFunctionType.Exp, bias=ebias[:], scale=1.0)
                est["act"] = max(est["act"] + EXP_OVH,
                                 est["pe"] + EXP_OVH) + width * ACTC
                exp_done[(h, ptbase)] = est["act"]
                sc_free[sidx[0] % 2] = est["act"]
                sidx[0] += 1
                for o in diags:
                    nc.vector.tensor_mul(
                        pt_t[h][:, o:o + P], pt_t[h][:, o:o + P], tri_t[:])
                    est["dve"] = max(est["dve"], est["act"]) + P * DVEC * 0.5 + DVE_OVH

            # ---- V fillers (tt pairs) ----
            v_done_est = [0.0] * NJ   # per tt, v_sb ready estimate

            def emit_vpair(pb):
                est["pe"] = max(est["pe"], arr[f"xv{pb}"], vps_free[0])
                for tl in range(2):
                    for kk in range(NKC):
                        nc.tensor.matmul(
                            vps[:, tl * CW:(tl + 1) * CW],
                            lhsT=xv_b[pb][:, kk, tl * P:(tl + 1) * P],
                            rhs=wvt[:, kk, :],
                            start=(kk == 0), stop=(kk == NKC - 1))
                    est["pe"] += NKC * CW * PEC
                nc.vector.tensor_copy(
                    v_sb[:, 2 * pb:2 * pb + 2, :, 0:HWID],
                    vps[:].rearrange("p (t h w) -> p t h w", t=2, h=HPC))
                est["dve"] = max(est["dve"], est["pe"]) + 512 * DVEC + COPY_OVH
                v_done_est[2 * pb] = v_done_est[2 * pb + 1] = est["dve"]
                vps_free[0] = est["dve"]

            # ---- O groups + epilogue + out DMA ----
            def o_gate_est(h, g):
                nb = NB_SLOT[h]
                ihi = min(NJ, (g + 1) * GSZ) - 1
                gate = 0.0
                for I in range(g * GSZ, ihi + 1):
                    col = _o_col(h, I, I)
                    # +400: DVE tri-mask of the diagonal block after the exp
                    gate = max(gate, exp_done[(h, win_base_of(h, col))] + 400.0)
                    gate = max(gate, v_done_est[I])
                return gate

            def emit_ogroup(h, g):
                nb = NB_SLOT[h]
                ilo, ihi = g * GSZ, min(NJ, (g + 1) * GSZ)
                est["pe"] = max(est["pe"], ops_free[0])
                for I in range(ilo, ihi):
                    Jlo = max(0, I - nb + 1)
                    for J in range(Jlo, I + 1):
                        col = _o_col(h, J, I)
                        nc.tensor.matmul(
                            ops[:, I - ilo, :],
                            lhsT=pt_t[h][:, col:col + P],
                            rhs=v_sb[:, J, h, :],
                            start=(J == Jlo), stop=(J == I))
                        est["pe"] += (HWID + 1) * PEC + 10
                # epilogue: batched reciprocal + per-I scale
                ng = ihi - ilo
                rec = rcp.tile([P, GSZ], F32, tag="rc", name=f"rc{h}_{g}")
                nc.vector.reciprocal(rec[:, 0:ng], ops[:, 0:ng, HWID])
                est["dve"] = max(est["dve"], est["pe"]) + ng * DVEC + DVE_OVH
                for I in range(ilo, ihi):
                    nc.vector.tensor_scalar_mul(
                        out_sb[h][:, I, :], ops[:, I - ilo, 0:HWID],
                        rec[:, I - ilo:I - ilo + 1])
                    est["dve"] += HWID * DVEC + DVE_OVH
                ops_free[0] = est["dve"]
                nc.sync.dma_start(
                    outs_d[h][:, ilo * HWID:ihi * HWID],
                    out_sb[h][:, ilo:ihi, :].rearrange("p a b -> p (a b)"))
                dma_est(f"out{h}_{g}", P * ng * HWID * 2)

            # ---- interleaved emission ----
            vq = list(range(NTB))                 # V pair blocks
            oq = [(h, g) for h in range(HPC) for g in range(NGRP)]
            wq_i = 0
            ACT_LEAD = 2600.0

            def o_ready():
                for i, (h, g) in enumerate(oq):
                    try:
                        if o_gate_est(h, g) <= est["pe"] + 50.0:
                            return i
                    except KeyError:
                        continue
                return None

            while wq_i < len(win_q) or vq or oq:
                did = False
                # keep ACT fed first
                if wq_i < len(win_q) and est["act"] - est["pe"] < ACT_LEAD:
                    emit_win(win_q[wq_i])
                    wq_i += 1
                    did = True
                elif vq and arr[f"xv{vq[0]}"] <= est["pe"] + 50.0:
                    emit_vpair(vq.pop(0))
                    did = True
                else:
                    i = o_ready()
                    if i is not None:
                        h, g = oq.pop(i)
                        emit_ogroup(h, g)
                        did = True
                if not did:
                    if wq_i < len(win_q):
                        emit_win(win_q[wq_i])
                        wq_i += 1
                    elif vq:
                        emit_vpair(vq.pop(0))
                        did = True
                    elif oq:
                        h, g = oq.pop(0)
                        emit_ogroup(h, g)

            ppO.release()
            ppV.release()
            scp.release()
            ptp.release()
            x2p.release()

    nc.compile()
    return nc


_NC = None


def _get_nc():
    global _NC
    if _NC is None:
        _NC = build_kernel()
    return _NC


def kernel(queries, keys, values, mask, Wq, Wk, Wv):
    B = queries.shape[0]
    bf16 = ml_dtypes.bfloat16
    asc = np.ascontiguousarray
    scale = 1.0 / np.sqrt(HWID)

    WqTs = (Wq.T * scale).astype(np.float32)
    WkT = Wk.T.astype(np.float32)
    WvT = Wv.T.astype(np.float32)
    xqs = [asc(queries[b].T).astype(bf16) for b in range(B)]
    xks = [asc(keys[b].T).astype(bf16) for b in range(B)]
    xvs = [asc(values[b].T).astype(bf16) for b in range(B)]

    slopes = (2.0 ** (-np.arange(1, H + 1) * (8.0 / H))).astype(np.float32)
    iv = np.arange(S, dtype=np.float32)
    tri_np = np.asarray(
        np.arange(P)[:, None] <= np.arange(P)[None, :], dtype=np.float32
    ).astype(bf16)

    nc = _get_nc()
    in_maps = []
    for c in range(8):
        b, g = divmod(c, 4)
        heads = GROUPS[g]
        cols = np.concatenate([np.arange(h * HWID, (h + 1) * HWID)
                               for h in heads])
        # aug[t][r][s][:]: t 0=q,1=k; r aug row; s head slot
        a = np.zeros((2, 2, HPC, S), np.float32)
        for sl in range(HPC):
            h = heads[sl]
            a[0, 0, sl] = slopes[h]
            a[0, 1, sl] = -slopes[h] * iv
            a[1, 0, sl] = iv
            a[1, 1, sl] = 1.0
        in_maps.append({
            "xq": xqs[b], "xk": xks[b], "xv": xvs[b],
            "wq": asc(WqTs[:, cols]).astype(bf16),
            "wk": asc(WkT[:, cols]).astype(bf16),
            "wv": asc(WvT[:, cols]).astype(bf16),
            "aug": a,
            "tri": tri_np,
        })

    res = run_bass_kernel_spmd(nc, in_maps, core_ids=list(range(8)))
    outp = np.empty((B, S, D), np.float32)
    for c in range(8):
        b, g = divmod(c, 4)
        heads = GROUPS[g]
        for sl, h in enumerate(heads):
            o = np.asarray(res.results[c][f"out{sl}"]).astype(np.float32)
            o = o.reshape(P, NJ, HWID).transpose(1, 0, 2).reshape(S, HWID)
            outp[b, :, h * HWID:(h + 1) * HWID] = o
    return outp
